# revision 11
# baseline (speedup 1.0000x reference)
"""MoE BatchedExperts kernel for 8 trn2 NeuronCores.

Strategy: expert parallelism with host-side top-k dispatch and exact load
balancing. Each token has TOP_K=2 nonzero routing weights; core c processes
a fixed per-core "slot structure" of expert token groups chosen so all
cores get ~N*K/E tokens (the hot experts are split across cores). All
matmuls run bf16 (1 row/cycle, same as fp32r, but half the DMA/SBUF and no
min-moving-dim constraint), PSUM accumulates fp32; measured end-to-end
rel err ~3e-3 vs the fp64 reference (gate 2e-2).

Per core, per group g (tokens gathered+transposed on host to xT [D, S_g]):
  h  = gelu(w0_g^T-tiles @ xT + b0)   [F-part, S_g]  tokens on moving dim
  yT = w1_g-tiles @ h                 [D-part, S_g]  tokens on moving dim
Host combines: out[idx] += r * yT.T rows; b1 folded in via routing @ b1.

Tokens stay on the PE moving dim in both phases so group sizes need no
128-padding. A few zero-filled warmup matmuls keep the PE busy (and ramp
its DVFS p-state) while the first input DMAs land.
"""

import numpy as np
import ml_dtypes

import concourse.bacc as bacc
import concourse.mybir as mybir
from concourse.tile import TileContext
from concourse.bass_utils import run_bass_kernel_spmd

F32 = mybir.dt.float32
BF16 = mybir.dt.bfloat16

N, D, E, F = 4096, 1024, 8, 2048
P = 128
KD = D // P            # 8  k-tiles for mm1 (contract D)
KF = F // P            # 16 k-tiles for mm2 (contract F)
DO = D // P            # 8  output d-tiles for mm2

_cache: dict[tuple, object] = {}


def _chunks_of(size: int) -> list[int]:
    """Split a group into near-equal moving-dim chunks <=512 (>=~250 keeps
    the per-matmul LDWEIGHTS (~97ns) hidden behind the previous matmul)."""
    n = -(-size // 512)
    base, rem = divmod(size, n)
    return [base + 1] * rem + [base] * (n - rem)


def build_program(sizes: tuple[int, ...]):
    """Bass program for one core: len(sizes) expert groups of fixed widths."""
    G = len(sizes)
    T = sum(sizes)
    goffs = [0, *np.cumsum(sizes).tolist()]
    chunks = [_chunks_of(s) for s in sizes]

    nc = bacc.Bacc("TRN2", target_bir_lowering=False, debug=False)
    xT = nc.dram_tensor("xT", [D, T], BF16, kind="ExternalInput")
    w0d = [nc.dram_tensor(f"w0_{g}", [D, F], BF16, kind="ExternalInput")
           for g in range(G)]
    w1d = [nc.dram_tensor(f"w1_{g}", [F, D], BF16, kind="ExternalInput")
           for g in range(G)]
    # b0 pre-arranged [128, G*KF] on the host (contiguous per-partition rows)
    b0 = nc.dram_tensor("b0", [P, G * KF], F32, kind="ExternalInput")
    yT = nc.dram_tensor("yT", [D, T], F32, kind="ExternalOutput")

    xT_r = xT.rearrange("(ko p) t -> p ko t", p=P)
    w0_r = [w.rearrange("(ko p) f -> p ko f", p=P) for w in w0d]
    w1_r = [w.rearrange("(ko p) d -> p ko d", p=P) for w in w1d]

    with TileContext(nc) as tc:
        with tc.tile_pool(name="const", bufs=1) as const, \
             tc.tile_pool(name="xpool", bufs=1) as xpool, \
             tc.tile_pool(name="hpool", bufs=1) as hpool, \
             tc.tile_pool(name="wpool", bufs=G + 1) as wpool, \
             tc.tile_pool(name="ypool", bufs=3) as ypool, \
             tc.tile_pool(name="psum", bufs=8, space="PSUM") as psum:

            # First group = widest first chunk: the PE's per-fo-tile
            # pace on a wide chunk (~1.65us) stays above the w0 stream
            # (~0.73-1.1us per fo tile), so phase 1 never starves after
            # the first matmul.
            gorder = sorted(range(G), key=lambda g: -chunks[g][0])

            # ---- PE warmup: zero matmuls bridge the first-input DMA
            # latency and ramp the PE's DVFS p-state ----
            warm = const.tile([P, 512], BF16, name="warm")
            nc.vector.memset(warm[:], 0.0)
            for i in range(5):
                pw = psum.tile([P, 512], F32, tag="ps", name=f"warm{i}")
                nc.tensor.matmul(pw, warm[:, 0:P], warm[:], start=True,
                                 stop=True)

            # ---- input DMAs ----
            # All in-flight DGE transfers share the DMA engines and ring
            # entries pump concurrently, so greedy issue starves the
            # critical path. Tiny SBUF->SBUF "gate" DMAs (traced AFTER the
            # transfer they wait on) hold a ring until a prerequisite
            # lands, making arrival order == need order. The scalar queue
            # carries only the first x chunk + b0, NEVER a gate: the gelu
            # ACTs run on the scalar engine and must not queue behind a
            # blocked DMA. Remaining x rides the otherwise-idle gpsimd
            # (SWDGE) queue.
            x_sb = xpool.tile([P, KD, T], BF16, name="x")
            b0_sb = const.tile([P, G * KF], F32, name="b0")
            gate = const.tile([P, 4], BF16, name="gate")
            w0_sb = [None] * G
            w1_sb = [None] * G
            for g in gorder:
                w0_sb[g] = wpool.tile([P, KD, F], BF16, tag="wbig",
                                      name=f"w0_{g}")
            g0 = gorder[0]
            c0 = chunks[g0][0]
            o0 = goffs[g0]

            # smallest first pieces: w0[g0] fo tile 0 and x0 k-sliced, so
            # the first real matmul can start ~2.5us after the rings open
            nc.sync.dma_start(w0_sb[g0][:, :, 0:128], w0_r[g0][:, :, 0:128])
            for ka, kb in ((0, 2), (2, 4), (4, KD)):
                nc.scalar.dma_start(x_sb[:, ka:kb, o0:o0 + c0],
                                    xT_r[:, ka:kb, o0:o0 + c0])
            nc.scalar.dma_start(b0_sb[:], b0[:, :])
            nc.sync.dma_start(w0_sb[g0][:, :, 128:256],
                              w0_r[g0][:, :, 128:256])
            # sync ring yields to x0, then streams the rest of w0
            nc.sync.dma_start(gate[:, 0:1],
                              x_sb[:, KD - 1, o0 + c0 - 1:o0 + c0])
            nc.sync.dma_start(w0_sb[g0][:, :, 256:1024],
                              w0_r[g0][:, :, 256:1024])
            nc.sync.dma_start(w0_sb[g0][:, :, 1024:2048],
                              w0_r[g0][:, :, 1024:2048])
            for g in gorder[1:]:
                for a in (0, 1024):
                    nc.sync.dma_start(w0_sb[g][:, :, a:a + 1024],
                                      w0_r[g][:, :, a:a + 1024])
            # remaining x on gpsimd, held until x0 is in
            if G > 1 or len(chunks[g0]) > 1:
                nc.gpsimd.dma_start(gate[:, 1:2],
                                    x_sb[:, KD - 1, o0 + c0 - 1:o0 + c0])
                for g in gorder:
                    off = goffs[g] + (c0 if g == g0 else 0)
                    for c in chunks[g][1 if g == g0 else 0:]:
                        nc.gpsimd.dma_start(x_sb[:, :, off:off + c],
                                            xT_r[:, :, off:off + c])
                        off += c
            # w1 after all w0 (phase-2 only), then the y stores
            nc.sync.dma_start(gate[:, 2:3],
                              w0_sb[gorder[-1]][:, KD - 1, F - 1:F])
            for g in gorder:
                w1_sb[g] = wpool.tile([P, KF, D], BF16, tag="wbig",
                                      name=f"w1_{g}")
                for a in (0, 512):
                    nc.sync.dma_start(w1_sb[g][:, :, a:a + 512],
                                      w1_r[g][:, :, a:a + 512])

            # h = gelu(x @ w0 + b0), [F-part, T-free], groups concatenated
            h_sb = hpool.tile([P, KF, T], BF16, name="h")

            # ---- phase 1: mm1 + gelu ----
            for g in gorder:
                off = goffs[g]
                for c in chunks[g]:
                    for fo in range(KF):
                        ps = psum.tile([P, 512], F32, tag="ps",
                                       name=f"ps1_{g}_{off}_{fo}")[:, :c]
                        for k in range(KD):
                            nc.tensor.matmul(
                                ps, w0_sb[g][:, k, fo * P:(fo + 1) * P],
                                x_sb[:, k, off:off + c],
                                start=(k == 0), stop=(k == KD - 1))
                        nc.scalar.activation(
                            h_sb[:, fo, off:off + c], ps,
                            mybir.ActivationFunctionType.Gelu,
                            bias=b0_sb[:, g * KF + fo:g * KF + fo + 1])
                    off += c

            # ---- phase 2: mm2 ----
            # same group order: the first mm2 group's h has long been
            # drained, and the final y store is the small tail chunk
            for g in gorder:
                off = goffs[g]
                for c in chunks[g]:
                    for do in range(DO):
                        ps2 = psum.tile([P, 512], F32, tag="ps",
                                        name=f"ps2_{g}_{off}_{do}")[:, :c]
                        for k in range(KF):
                            nc.tensor.matmul(
                                ps2, w1_sb[g][:, k, do * P:(do + 1) * P],
                                h_sb[:, k, off:off + c],
                                start=(k == 0), stop=(k == KF - 1))
                        y_sb = ypool.tile([P, 512], F32, tag="y",
                                          name=f"y_{g}_{off}_{do}")[:, :c]
                        nc.vector.tensor_copy(y_sb, ps2)
                        nc.sync.dma_start(
                            yT[do * P:(do + 1) * P, off:off + c], y_sb)
                    off += c

    nc.compile()
    return nc


def _plan(counts):
    """Choose per-core slot sizes (S1, S2) and assign expert token pieces.

    Minimizes T = S1 + S2 such that the 8 experts can be covered by 8
    pieces of size <= S1 plus 8 of size <= S2 (pieces of one expert may
    live on different cores). Falls back to one-slot-per-core (pure expert
    parallelism) if the search fails.
    """
    cmax = int(max(counts))
    order = sorted(range(E), key=lambda e: -counts[e])
    csort = [int(counts[e]) for e in order]

    def assign(S1, S2):
        from functools import lru_cache

        @lru_cache(maxsize=None)
        def feas(i, a, b):
            if i == len(csort):
                return ()
            c = csort[i]
            opts = []
            if c <= S1: opts.append((1, 0))
            if c <= S2: opts.append((0, 1))
            if c <= 2 * S2: opts.append((0, 2))
            if c <= S1 + S2: opts.append((1, 1))
            if c <= 2 * S1: opts.append((2, 0))
            if c <= S1 + 2 * S2: opts.append((1, 2))
            if c <= 2 * S1 + S2: opts.append((2, 1))
            opts.sort(key=lambda uv: (uv[0] + uv[1], S1 * uv[0] + S2 * uv[1]))
            for u, v in opts:
                if u <= a and v <= b:
                    rest = feas(i + 1, a - u, b - v)
                    if rest is not None:
                        return ((u, v),) + rest
            return None

        return feas(0, 8, 8)

    best = None
    for T in range(-(-N * 2 // E), cmax + 1):
        for S1 in range(-(-T // 2), T):
            S2 = T - S1
            sol = assign(S1, S2)
            if sol is not None:
                best = (S1, S2, sol)
                break
        if best:
            break
    if best is None:
        sizes = (cmax,)
        cores = [[(e, 0, int(counts[e]))] for e in range(E)]
        return sizes, cores

    S1, S2, sol = best
    s1_pieces, s2_pieces = [], []
    for i, (u, v) in enumerate(sol):
        e, c = order[i], csort[i]
        caps = [S1] * u + [S2] * v
        lo_ = 0
        for j, cap in enumerate(caps):
            take = min(cap, c - lo_)
            # ensure later pieces aren't left with more than they can hold
            take = max(take, c - lo_ - sum(caps[j + 1:]))
            (s1_pieces if cap == S1 else s2_pieces).append((e, lo_, take))
            lo_ += take
    while len(s1_pieces) < 8:
        s1_pieces.append((0, 0, 0))
    while len(s2_pieces) < 8:
        s2_pieces.append((0, 0, 0))
    sizes = (S1, S2)
    cores = [[s1_pieces[i], s2_pieces[i]] for i in range(8)]
    return sizes, cores


def kernel(x, routing_tensor, w0, b0, w1, b1):
    x = np.ascontiguousarray(np.asarray(x, dtype=np.float32))
    routing = np.asarray(routing_tensor, dtype=np.float32)
    w0 = np.asarray(w0, dtype=np.float32)
    b0 = np.asarray(b0, dtype=np.float32)
    w1 = np.asarray(w1, dtype=np.float32)
    b1 = np.asarray(b1, dtype=np.float32)

    idx = [np.nonzero(routing[:, e])[0] for e in range(E)]
    counts = [len(i) for i in idx]
    sizes, cores = _plan(counts)
    G = len(sizes)
    T = sum(sizes)
    goffs = np.concatenate([[0], np.cumsum(sizes)])

    nc = _cache.get(sizes)
    if nc is None:
        nc = _cache[sizes] = build_program(sizes)

    w0_bf = [np.ascontiguousarray(w0[e], dtype=ml_dtypes.bfloat16)
             for e in range(E)]
    w1_bf = [np.ascontiguousarray(w1[e], dtype=ml_dtypes.bfloat16)
             for e in range(E)]
    b0_cols = [np.ascontiguousarray(b0[e, 0].reshape(KF, P).T)
               for e in range(E)]

    in_maps = []
    for core in cores:
        xTc = np.zeros((D, T), dtype=ml_dtypes.bfloat16)
        b0c = np.empty((P, G * KF), dtype=np.float32)
        m = {"xT": xTc, "b0": b0c}
        for g, (e, lo, cnt) in enumerate(core):
            tok = idx[e][lo:lo + cnt]
            xTc[:, goffs[g]:goffs[g] + cnt] = \
                x[tok].T.astype(ml_dtypes.bfloat16)
            b0c[:, g * KF:(g + 1) * KF] = b0_cols[e]
            m[f"w0_{g}"] = w0_bf[e]
            m[f"w1_{g}"] = w1_bf[e]
        in_maps.append(m)

    res = run_bass_kernel_spmd(nc, in_maps, core_ids=list(range(8)))

    # combine: out = routing @ b1 + sum of r_e-scaled group outputs
    out = routing @ b1[:, 0, :]
    for ci, core in enumerate(cores):
        yT = res.results[ci]["yT"]
        for g, (e, lo, cnt) in enumerate(core):
            if cnt == 0:
                continue
            tok = idx[e][lo:lo + cnt]
            out[tok] += routing[tok, e:e + 1] * yT[:, goffs[g]:goffs[g] + cnt].T
    return out.astype(np.float32)
